# revision 24
# baseline (speedup 1.0000x reference)
"""Trainium2 Bass kernel for nn_KANSplineLayer (KAN spline layer, 8-core SPMD).

Math rewrite (validated to ~3.8e-3 L2 rel err vs reference):
  reference: out = silu(BN_b(x @ Wb)) + BN_s(basis(minmax(x)) @ Ws.T)
  with 9 wide triangle-basis functions per input feature.

  The spline g(z) is continuous piecewise-linear on t = 4*z in [0,4) with
  breakpoints {1,2,3}, so it equals a linear combination of
  {t, relu(t-1), relu(t-2), relu(t-3), 1}.  The global per-feature min/max
  (a reduction over ALL rows, identical on every shard) is computed on the
  host, so the device needs no collective, and the host ships the centered
  plane tc = (x - gmin)*s4 - 2 pre-transposed in fp16.

  Since t is affine in x, the t-term of the spline and the base GEMM merge
  into ONE moving operand [W_t | Wb/s4] of width 512.  Constants fold into
  the rank-1 ones GEMM (pre-silu base bias) or a host-side add (spline
  const).  The r2/r3 relu planes are sparse-ish and small-valued, so they
  are shipped pre-quantized in fp8e4 with fp8 weights and contracted with
  DoubleRow matmuls (both 128-feature blocks in ONE half-rate matmul);
  r1 carries the large values and stays fp16 (computed on-device by DVE).

Sharding: data-parallel over rows (batch*H*W = 32768 -> 4096 rows/core).

Device pipeline per core (single phase, PE-bound):
  dual-queue DMA (qSP: tc block0 + r2/r3 planes, qAct: tc block1 + weights
  + output stores) -> DVE r1 plane (fp16 4x mode) -> per 128-row tile:
  7 accumulating matmuls into one PSUM bank [spline | base], ACT silu on
  the base half, DVE add, fp16 DMA out per 512-row group.
"""
import numpy as np

import concourse.bacc as bacc
import concourse.bass as bass
import concourse.tile as tile
from concourse import mybir
from concourse.bass_utils import run_bass_kernel_spmd

# ---- problem constants (hardcoded; kernel.py must be self-contained) ----
IN_F, OUT_F = 256, 256
K_KNOTS = 9
EPS_MINMAX = 1e-7
EPS_BN = 1e-3
B, H, W = 32, 32, 32
N_TOTAL = B * H * W            # 32768 rows
N_CORES = 8
N_SHARD = N_TOTAL // N_CORES   # 4096 rows per core
CH = 1024                      # rows per plane chunk
N_CHUNKS = N_SHARD // CH       # 4
J_PER_CH = CH // 128           # 8

F32 = mybir.dt.float32
DT = mybir.dt.float16
F8 = mybir.dt.float8e4
NP_DT = np.float16
NP_F8 = mybir.dt.np(mybir.dt.float8e4)
_ACT = mybir.ActivationFunctionType.Silu   # overridable for CoreSim debug


def _host_prep(x, base_weight, spline_weight, spline_scaler,
               bn_base_gamma, bn_base_beta, bn_base_mean, bn_base_var,
               bn_spline_gamma, bn_spline_beta, bn_spline_mean, bn_spline_var):
    """Fold BN + rewrite spline into relu-plane weights; global min/max and
    the centered normalized plane tc are computed here (host), fp64 weights."""
    f64 = np.float64
    xf = np.ascontiguousarray(np.asarray(x, np.float32)).reshape(N_TOTAL, IN_F)

    w = np.asarray(spline_weight, f64) * np.asarray(spline_scaler, f64)[:, :, None]
    knots = np.linspace(-1.0, 1.0, K_KNOTS).astype(f64)
    jg = np.arange(5, dtype=f64) / 4.0
    tri = np.maximum(0.0, 1.0 - np.abs(jg[None, :] - knots[:, None]))   # [k, j]
    G = np.einsum('oik,kj->oij', w, tri)                                # [o,i,5]
    a_s = np.asarray(bn_spline_gamma, f64) / np.sqrt(np.asarray(bn_spline_var, f64) + EPS_BN)
    b_s = np.asarray(bn_spline_beta, f64) - a_s * np.asarray(bn_spline_mean, f64)
    G = G * a_s[:, None, None]
    W_t = (G[:, :, 1] - G[:, :, 0]).T                                   # [i,o] t-coeff
    H1 = (G[:, :, 2] - 2 * G[:, :, 1] + G[:, :, 0]).T
    H2 = (G[:, :, 3] - 2 * G[:, :, 2] + G[:, :, 1]).T
    H3 = (G[:, :, 4] - 2 * G[:, :, 3] + G[:, :, 2]).T
    C_s = G[:, :, 0].sum(axis=1) + b_s                                  # [o]

    a_b = np.asarray(bn_base_gamma, f64) / np.sqrt(np.asarray(bn_base_var, f64) + EPS_BN)
    b_b = np.asarray(bn_base_beta, f64) - a_b * np.asarray(bn_base_mean, f64)
    Wb = np.asarray(base_weight, f64) * a_b[None, :]                    # [i,o]

    gmin = xf.min(axis=0).astype(f64)
    gmax = xf.max(axis=0).astype(f64)
    s4 = 4.0 / (gmax - gmin + EPS_MINMAX)      # t = (x-gmin)*s4 in [0,4)

    # centered plane tc = t - 2: spline t-term gains const 2*sum(W_t);
    # base x = tc/s4 + (gmin + 2/s4) folds into Wb/s4 + bias shift.
    C_host = (C_s + 2.0 * W_t.sum(axis=0)).astype(np.float32)           # host-side add
    Wbp = Wb / s4[:, None]
    b_dev = b_b + ((gmin + 2.0 / s4)[:, None] * Wb).sum(axis=0)         # pre-silu bias

    tc = ((xf.astype(f64) - gmin) * s4 - 2.0).astype(NP_DT)             # [N, in]

    W_lin = np.concatenate([W_t, Wbp], axis=1)                          # [i, 512]
    w_lin = np.stack([W_lin[b * 128:(b + 1) * 128] for b in range(2)]).astype(NP_DT)
    w_r1 = np.stack([H1[b * 128:(b + 1) * 128] for b in range(2)]).astype(NP_DT)
    wr8 = np.stack([                                                    # [m,b,128,256]
        np.stack([Hm[b * 128:(b + 1) * 128] for b in range(2)])
        for Hm in (H2, H3)]).astype(NP_F8)
    bias_row = b_dev.astype(NP_DT)[None, :]                             # [1,256]
    return tc, w_lin, w_r1, wr8, bias_row, C_host


def _build_bass():
    nc = bacc.Bacc(num_devices=N_CORES)
    tc_sh = nc.declare_dram_parameter("tc_sh", [2, 128, N_SHARD], DT, isOutput=False)
    r23_sh = nc.declare_dram_parameter("r23_sh", [2, 2, 128, N_SHARD], F8, isOutput=False)
    w_lin_d = nc.declare_dram_parameter("w_lin", [2, 128, 512], DT, isOutput=False)
    w_r1_d = nc.declare_dram_parameter("w_r1", [2, 128, 256], DT, isOutput=False)
    wr8_d = nc.declare_dram_parameter("wr8", [2, 128, 2, 256], F8, isOutput=False)
    bias_d = nc.declare_dram_parameter("bias_row", [1, 256], DT, isOutput=False)
    out_sh = nc.declare_dram_parameter("out_sh", [N_SHARD, OUT_F], DT, isOutput=True)

    from contextlib import ExitStack
    with tile.TileContext(nc) as tc_ctx, ExitStack() as es:
        cons = es.enter_context(tc_ctx.tile_pool(name="cons", bufs=1))
        planes_p = es.enter_context(tc_ctx.tile_pool(name="planes", bufs=2))
        psM = es.enter_context(tc_ctx.tile_pool(name="psM", bufs=3, space="PSUM"))
        psW = es.enter_context(tc_ctx.tile_pool(name="psW", bufs=1, space="PSUM"))
        outp = es.enter_context(tc_ctx.tile_pool(name="outp", bufs=3))

        # ---- inputs split across the two HWDGE queues so they transfer in
        # parallel: qSP (sync) carries tc block0 + fp8 r23 planes; qAct
        # (scalar) carries tc block1 + weights, and later output stores.
        # Early chunks are small so the PE can start ASAP. ----
        CHUNKS = [(0, 256), (256, 256), (512, 512), (1024, 1024), (2048, 2048)]
        xt = cons.tile([128, 2, N_SHARD], DT, name="xt")
        r23t = cons.tile([128, 2, 2, N_SHARD], F8, name="r23t")

        def dma_tc(cs):
            nc.sync.dma_start(out=xt[:, 0, cs], in_=tc_sh[0, :, cs])
            nc.scalar.dma_start(out=xt[:, 1, cs], in_=tc_sh[1, :, cs])

        def dma_r23(cs):
            # coarse pieces only: fine slices make 256B-segment descriptor
            # storms that stall the whole queue
            nc.sync.dma_start(
                out=r23t[:, :, :, cs],
                in_=r23_sh[:, :, :, cs].rearrange("m b p n -> p m b n"))

        # tiny head chunks land fast on the HWDGE queues; all weights go via
        # GPSIMD software DGE — a third, otherwise-idle DMA issue path
        wlin_sb = cons.tile([128, 2, 512], DT, name="wlin_sb")
        nc.gpsimd.dma_start(out=wlin_sb[:], in_=w_lin_d.rearrange("b p n -> p b n"))
        w1_sb = cons.tile([128, 2, 256], DT, name="w1_sb")
        nc.gpsimd.dma_start(out=w1_sb[:], in_=w_r1_d.rearrange("b p n -> p b n"))
        bias_sb = cons.tile([1, 256], DT, name="bias_sb")
        nc.gpsimd.dma_start(out=bias_sb[:], in_=bias_d[:])
        wr8_sb = cons.tile([128, 2, 2, 256], F8, name="wr8_sb")
        nc.gpsimd.dma_start(out=wr8_sb[:], in_=wr8_d.rearrange("m p b n -> p m b n"))
        dma_tc(slice(0, 256))
        dma_tc(slice(256, 512))
        dma_r23(slice(0, 512))
        dma_tc(slice(512, 1024))
        dma_r23(slice(512, 1024))
        dma_tc(slice(1024, 2048))
        dma_r23(slice(1024, 2048))
        dma_tc(slice(2048, 4096))
        dma_r23(slice(2048, 4096))
        ones = cons.tile([1, 128], DT, name="ones")
        nc.vector.memset(ones[:], 1.0)

        # ---- PE pre-warm: wide dummy matmuls (N=1024, minimal LDWEIGHTS
        # bubbles) so the HAM clock gate reaches 8/8 before the first real
        # matmul and the PE is never idle while chunk 0 is in flight ----
        zrow = cons.tile([1, 512], DT, name="zrow")
        nc.vector.memset(zrow[:], 0.0)
        ps_w = psW.tile([128, 512], F32, name="warm")
        for _ in range(9):
            nc.tensor.matmul(ps_w[:], ones[:], zrow[:],
                             start=True, stop=True, skip_group_check=True)

        def mm_lin(ps, h, r0, b, start):
            nc.tensor.matmul(               # merged [W_t | Wb']: full bank
                ps[:, h, 0:512], xt[:, b, r0:r0 + 128], wlin_sb[:, b, :],
                start=start, stop=False, skip_group_check=True)

        def mm_bias(ps, h):
            nc.tensor.matmul(               # rank-1 pre-silu base bias
                ps[:, h, 256:512], ones[:], bias_sb[:],
                start=False, stop=False, skip_group_check=True)

        def mm_r1(ps, h, r1pl, js, b, stop):
            nc.tensor.matmul(
                ps[:, h, 0:256], r1pl[:, b, js], w1_sb[:, b, :],
                start=False, stop=stop, skip_group_check=True)

        def mm_dr(ps, h, rs, mi, stop):
            # r2/r3 fp8 DoubleRow: both feature blocks in one matmul
            nc.tensor.matmul(
                ps[:, h, 0:256], r23t[:, mi, :, rs], wr8_sb[:, mi, :, :],
                perf_mode=mybir.MatmulPerfMode.DoubleRow,
                start=False, stop=stop, skip_group_check=True)

        def do_pair(ps, s, q, pp, r1pl, defer_dr):
            hdat = []
            for h in range(2):
                j = q * 4 + pp * 2 + h
                r0 = s + j * 128
                js = slice(j * 128, (j + 1) * 128)      # chunk-local
                rs = slice(r0, r0 + 128)                # shard-absolute
                hdat.append(rs)
                if defer_dr:
                    # early chunks: defer DoubleRow MMs until r23 lands
                    mm_lin(ps, h, r0, 0, True)
                    mm_lin(ps, h, r0, 1, False)
                    mm_bias(ps, h)
                    mm_r1(ps, h, r1pl, js, 0, False)
                    mm_r1(ps, h, r1pl, js, 1, False)
                else:
                    # steady state: interleave the two DoubleRow MMs
                    # (213ns LDWEIGHTS) under the N=512 linear MMs
                    mm_lin(ps, h, r0, 0, True)
                    mm_dr(ps, h, rs, 0, False)
                    mm_lin(ps, h, r0, 1, False)
                    mm_dr(ps, h, rs, 1, False)
                    mm_bias(ps, h)
                    mm_r1(ps, h, r1pl, js, 0, False)
                    mm_r1(ps, h, r1pl, js, 1, True)
            if defer_dr:
                for h in range(2):
                    mm_dr(ps, h, hdat[h], 0, False)
                    mm_dr(ps, h, hdat[h], 1, True)

        def epilogue(oq, sl, ps):
            nc.scalar.activation(
                out=oq[:, sl, :], in_=ps[:, :, 256:512], func=_ACT)
            nc.vector.tensor_tensor(
                out=oq[:, sl, :], in0=oq[:, sl, :], in1=ps[:, :, 0:256],
                op=mybir.AluOpType.add)

        for ci, (s, n) in enumerate(CHUNKS):
            cs = slice(s, s + n)
            # r1 = relu(tc + 1): one fused DVE op per chunk (fp16 4x mode)
            r1pl = planes_p.tile([128, 2, n], DT, tag=f"r1_{n}",
                                 name=f"r1_{ci}")
            nc.vector.tensor_scalar(
                out=r1pl[:], in0=xt[:, :, cs],
                scalar1=1.0, scalar2=0.0,
                op0=mybir.AluOpType.add, op1=mybir.AluOpType.max)
            last_chunk = ci == len(CHUNKS) - 1
            if n == 256:
                # tiny head chunk: one PSUM pair, pair-sized store
                ps = psM.tile([128, 2, 512], F32)
                do_pair(ps, s, 0, 0, r1pl, defer_dr=True)
                oq = outp.tile([128, 2, OUT_F], DT, tag="oq2")
                epilogue(oq, slice(0, 2), ps)
                nc.scalar.dma_start(
                    out=out_sh[s:s + 256, :].rearrange("(g p) n -> p g n", g=2),
                    in_=oq[:])
                continue
            n_quads = n // 512
            for q in range(n_quads):
                # four 128-row tiles -> one SBUF out tile; PSUM pairs
                oq = outp.tile([128, 4, OUT_F], DT, tag="oq4")
                q0 = s + q * 512
                last_quad = last_chunk and q == n_quads - 1
                for pp in range(2):
                    ps = psM.tile([128, 2, 512], F32)
                    do_pair(ps, s, q, pp, r1pl, defer_dr=False)
                    sl = slice(pp * 2, pp * 2 + 2)
                    if last_quad and pp == 1:
                        # final pair: single-tile epilogues + stores so the
                        # post-matmul tail is as short as possible
                        for h in range(2):
                            shl = slice(pp * 2 + h, pp * 2 + h + 1)
                            nc.scalar.activation(
                                out=oq[:, shl, :], in_=ps[:, h:h + 1, 256:512],
                                func=_ACT)
                            nc.vector.tensor_tensor(
                                out=oq[:, shl, :], in0=oq[:, shl, :],
                                in1=ps[:, h:h + 1, 0:256],
                                op=mybir.AluOpType.add)
                            r0t = q0 + pp * 256 + h * 128
                            nc.scalar.dma_start(
                                out=out_sh[r0t:r0t + 128, :],
                                in_=oq[:, pp * 2 + h, :])
                        continue
                    epilogue(oq, sl, ps)
                    if last_quad:
                        # split the final store per pair to shorten the tail
                        nc.scalar.dma_start(
                            out=out_sh[q0 + pp * 256:q0 + pp * 256 + 256, :]
                                .rearrange("(g p) n -> p g n", g=2),
                            in_=oq[:, sl, :])
                if not last_quad:
                    nc.scalar.dma_start(
                        out=out_sh[q0:q0 + 512, :].rearrange("(g p) n -> p g n", g=4),
                        in_=oq[:])
    nc.compile()
    return nc


_CACHE = {}


def make_in_maps(inputs):
    tc, w_lin, w_r1, wr8, bias_row, C_host = _host_prep(**inputs)
    _CACHE["C_host"] = C_host
    maps = []
    for c in range(N_CORES):
        sh = tc[c * N_SHARD:(c + 1) * N_SHARD]          # [4096, 256]
        tct = np.ascontiguousarray(sh.T).reshape(2, 128, N_SHARD)
        t32 = tct.astype(np.float32)
        r23 = np.stack([np.maximum(t32, 0.0),           # r2 = relu(t-2)
                        np.maximum(t32 - 1.0, 0.0)])    # r3 = relu(t-3)
        maps.append({
            "tc_sh": tct, "r23_sh": r23.astype(NP_F8),
            "w_lin": w_lin, "w_r1": w_r1, "wr8": np.ascontiguousarray(
                wr8.transpose(0, 2, 1, 3)),             # [m,128,b,256]
            "bias_row": bias_row,
        })
    return maps


def kernel(**inputs):
    if "nc" not in _CACHE:
        _CACHE["nc"] = _build_bass()
    nc = _CACHE["nc"]
    in_maps = make_in_maps(inputs)
    res = run_bass_kernel_spmd(nc, in_maps, list(range(N_CORES)))
    out = np.concatenate([res.results[c]["out_sh"] for c in range(N_CORES)], axis=0)
    out = out.astype(np.float32) + _CACHE["C_host"][None, :]
    return out.reshape(B, H, W, OUT_F)


# revision 31
# speedup vs baseline: 1.0716x; 1.0716x over previous
"""Trainium2 Bass kernel for nn_KANSplineLayer (KAN spline layer, 8-core SPMD).

Math rewrite (validated to ~3.8e-3 L2 rel err vs reference):
  reference: out = silu(BN_b(x @ Wb)) + BN_s(basis(minmax(x)) @ Ws.T)
  with 9 wide triangle-basis functions per input feature.

  The spline g(z) is continuous piecewise-linear on t = 4*z in [0,4) with
  breakpoints {1,2,3}, so it equals a linear combination of
  {t, relu(t-1), relu(t-2), relu(t-3), 1}.  The global per-feature min/max
  (a reduction over ALL rows, identical on every shard) is computed on the
  host, so the device needs no collective, and the host ships the centered
  plane tc = (x - gmin)*s4 - 2 pre-transposed in fp16.

  Since t is affine in x, the t-term of the spline and the base GEMM merge
  into ONE moving operand [W_t | Wb/s4] of width 512.  Constants fold into
  the rank-1 ones GEMM (pre-silu base bias) or a host-side add (spline
  const).  The r2/r3 relu planes are sparse-ish and small-valued, so they
  are shipped pre-quantized in fp8e4 with fp8 weights and contracted with
  DoubleRow matmuls (both 128-feature blocks in ONE half-rate matmul);
  r1 carries the large values and stays fp16 (computed on-device by DVE).

Sharding: data-parallel over rows (batch*H*W = 32768 -> 4096 rows/core).

Device pipeline per core (single phase, PE-bound):
  dual-queue DMA (qSP: tc block0 + r2/r3 planes, qAct: tc block1 + weights
  + output stores) -> DVE r1 plane (fp16 4x mode) -> per 128-row tile:
  7 accumulating matmuls into one PSUM bank [spline | base], ACT silu on
  the base half, DVE add, fp16 DMA out per 512-row group.
"""
import numpy as np

import concourse.bacc as bacc
import concourse.bass as bass
import concourse.tile as tile
from concourse import mybir
from concourse.bass_utils import run_bass_kernel_spmd

# ---- problem constants (hardcoded; kernel.py must be self-contained) ----
IN_F, OUT_F = 256, 256
K_KNOTS = 9
EPS_MINMAX = 1e-7
EPS_BN = 1e-3
B, H, W = 32, 32, 32
N_TOTAL = B * H * W            # 32768 rows
N_CORES = 8
N_SHARD = N_TOTAL // N_CORES   # 4096 rows per core
CH = 1024                      # rows per plane chunk
N_CHUNKS = N_SHARD // CH       # 4
J_PER_CH = CH // 128           # 8

F32 = mybir.dt.float32
DT = mybir.dt.float16
F8 = mybir.dt.float8e4
NP_DT = np.float16
NP_F8 = mybir.dt.np(mybir.dt.float8e4)
_ACT = mybir.ActivationFunctionType.Silu   # overridable for CoreSim debug


def _host_prep(x, base_weight, spline_weight, spline_scaler,
               bn_base_gamma, bn_base_beta, bn_base_mean, bn_base_var,
               bn_spline_gamma, bn_spline_beta, bn_spline_mean, bn_spline_var):
    """Fold BN + rewrite spline into relu-plane weights; global min/max and
    the centered normalized plane tc are computed here (host), fp64 weights."""
    f64 = np.float64
    xf = np.ascontiguousarray(np.asarray(x, np.float32)).reshape(N_TOTAL, IN_F)

    w = np.asarray(spline_weight, f64) * np.asarray(spline_scaler, f64)[:, :, None]
    knots = np.linspace(-1.0, 1.0, K_KNOTS).astype(f64)
    jg = np.arange(5, dtype=f64) / 4.0
    tri = np.maximum(0.0, 1.0 - np.abs(jg[None, :] - knots[:, None]))   # [k, j]
    G = np.einsum('oik,kj->oij', w, tri)                                # [o,i,5]
    a_s = np.asarray(bn_spline_gamma, f64) / np.sqrt(np.asarray(bn_spline_var, f64) + EPS_BN)
    b_s = np.asarray(bn_spline_beta, f64) - a_s * np.asarray(bn_spline_mean, f64)
    G = G * a_s[:, None, None]
    W_t = (G[:, :, 1] - G[:, :, 0]).T                                   # [i,o] t-coeff
    H1 = (G[:, :, 2] - 2 * G[:, :, 1] + G[:, :, 0]).T
    H2 = (G[:, :, 3] - 2 * G[:, :, 2] + G[:, :, 1]).T
    H3 = (G[:, :, 4] - 2 * G[:, :, 3] + G[:, :, 2]).T
    C_s = G[:, :, 0].sum(axis=1) + b_s                                  # [o]

    a_b = np.asarray(bn_base_gamma, f64) / np.sqrt(np.asarray(bn_base_var, f64) + EPS_BN)
    b_b = np.asarray(bn_base_beta, f64) - a_b * np.asarray(bn_base_mean, f64)
    Wb = np.asarray(base_weight, f64) * a_b[None, :]                    # [i,o]

    gmin = xf.min(axis=0).astype(f64)
    gmax = xf.max(axis=0).astype(f64)
    s4 = 4.0 / (gmax - gmin + EPS_MINMAX)      # t = (x-gmin)*s4 in [0,4)

    # centered plane tc = t - 2: spline t-term gains const 2*sum(W_t);
    # base x = tc/s4 + (gmin + 2/s4) folds into Wb/s4 + bias shift.
    C_host = (C_s + 2.0 * W_t.sum(axis=0)).astype(np.float32)           # host-side add
    Wbp = Wb / s4[:, None]
    b_dev = b_b + ((gmin + 2.0 / s4)[:, None] * Wb).sum(axis=0)         # pre-silu bias

    tc = ((xf.astype(f64) - gmin) * s4 - 2.0).astype(NP_DT)             # [N, in]

    # one packed fp16 weight tensor: [W_t | Wb' | H1] per feature block
    W_lin = np.concatenate([W_t, Wbp, H1], axis=1)                      # [i, 768]
    w_pack = np.stack([W_lin[b * 128:(b + 1) * 128] for b in range(2)]).astype(NP_DT)
    wr8 = np.stack([                                                    # [m,b,128,256]
        np.stack([Hm[b * 128:(b + 1) * 128] for b in range(2)])
        for Hm in (H2, H3)]).astype(NP_F8)
    bias_row = b_dev.astype(NP_DT)[None, :]                             # [1,256]
    return tc, w_pack, wr8, bias_row, C_host


def _build_bass():
    nc = bacc.Bacc(num_devices=N_CORES)
    tc_sh = nc.declare_dram_parameter("tc_sh", [2, 128, N_SHARD], DT, isOutput=False)
    r23_sh = nc.declare_dram_parameter("r23_sh", [2, 2, 128, N_SHARD], F8, isOutput=False)
    w_pack_d = nc.declare_dram_parameter("w_pack", [2, 128, 768], DT, isOutput=False)
    wr8_d = nc.declare_dram_parameter("wr8", [2, 128, 2, 256], F8, isOutput=False)
    bias_d = nc.declare_dram_parameter("bias_row", [1, 256], DT, isOutput=False)
    out_sh = nc.declare_dram_parameter("out_sh", [N_SHARD, OUT_F], DT, isOutput=True)

    from contextlib import ExitStack
    with tile.TileContext(nc) as tc_ctx, ExitStack() as es:
        cons = es.enter_context(tc_ctx.tile_pool(name="cons", bufs=1))
        planes_p = es.enter_context(tc_ctx.tile_pool(name="planes", bufs=2))
        psM = es.enter_context(tc_ctx.tile_pool(name="psM", bufs=3, space="PSUM"))
        psW = es.enter_context(tc_ctx.tile_pool(name="psW", bufs=1, space="PSUM"))
        outp = es.enter_context(tc_ctx.tile_pool(name="outp", bufs=3))

        # ---- inputs split across the two HWDGE queues so they transfer in
        # parallel: qSP (sync) carries tc block0 + fp8 r23 planes; qAct
        # (scalar) carries tc block1 + weights, and later output stores.
        # Early chunks are small so the PE can start ASAP. ----
        CHUNKS = [(0, 256), (256, 256), (512, 512), (1024, 1024), (2048, 2048)]
        xt = cons.tile([128, 2, N_SHARD], DT, name="xt")
        r23t = cons.tile([128, 2, 2, N_SHARD], F8, name="r23t")

        def dma_tc(cs):
            nc.sync.dma_start(out=xt[:, 0, cs], in_=tc_sh[0, :, cs])
            nc.scalar.dma_start(out=xt[:, 1, cs], in_=tc_sh[1, :, cs])

        def dma_r23(cs):
            # coarse pieces only: fine slices make 256B-segment descriptor
            # storms that stall the whole queue
            nc.sync.dma_start(
                out=r23t[:, :, :, cs],
                in_=r23_sh[:, :, :, cs].rearrange("m b p n -> p m b n"))

        # tiny head chunks land fast; packed weights interleave on qAct
        dma_tc(slice(0, 256))
        wpack_sb = cons.tile([128, 2, 768], DT, name="wpack_sb")
        nc.scalar.dma_start(out=wpack_sb[:], in_=w_pack_d.rearrange("b p n -> p b n"))
        dma_tc(slice(256, 512))
        bias_sb = cons.tile([1, 256], DT, name="bias_sb")
        nc.scalar.dma_start(out=bias_sb[:], in_=bias_d[:])
        wr8_sb = cons.tile([128, 2, 2, 256], F8, name="wr8_sb")
        nc.scalar.dma_start(out=wr8_sb[:], in_=wr8_d.rearrange("m p b n -> p m b n"))
        dma_r23(slice(0, 512))
        dma_tc(slice(512, 1024))
        dma_r23(slice(512, 1024))
        dma_tc(slice(1024, 2048))
        dma_r23(slice(1024, 2048))
        dma_tc(slice(2048, 4096))
        dma_r23(slice(2048, 4096))
        ones = cons.tile([1, 128], DT, name="ones")
        nc.vector.memset(ones[:], 1.0)

        # ---- PE pre-warm: wide dummy matmuls (N=1024, minimal LDWEIGHTS
        # bubbles) so the HAM clock gate reaches 8/8 before the first real
        # matmul and the PE is never idle while chunk 0 is in flight ----
        zrow = cons.tile([1, 512], DT, name="zrow")
        nc.vector.memset(zrow[:], 0.0)
        ps_w = psW.tile([128, 512], F32, name="warm")
        for _ in range(9):
            nc.tensor.matmul(ps_w[:], ones[:], zrow[:],
                             start=True, stop=True, skip_group_check=True)

        def mm_lin(ps, h, r0, b, start):
            nc.tensor.matmul(               # merged [W_t | Wb']: full bank
                ps[:, h, 0:512], xt[:, b, r0:r0 + 128], wpack_sb[:, b, 0:512],
                start=start, stop=False, skip_group_check=True)

        def mm_bias(ps, h):
            nc.tensor.matmul(               # rank-1 pre-silu base bias
                ps[:, h, 256:512], ones[:], bias_sb[:],
                start=False, stop=False, skip_group_check=True)

        def mm_r1(ps, h, r1pl, js, b, stop):
            nc.tensor.matmul(
                ps[:, h, 0:256], r1pl[:, b, js], wpack_sb[:, b, 512:768],
                start=False, stop=stop, skip_group_check=True)

        def mm_dr(ps, h, rs, mi, stop):
            # r2/r3 fp8 DoubleRow: both feature blocks in one matmul
            nc.tensor.matmul(
                ps[:, h, 0:256], r23t[:, mi, :, rs], wr8_sb[:, mi, :, :],
                perf_mode=mybir.MatmulPerfMode.DoubleRow,
                start=False, stop=stop, skip_group_check=True)

        def do_pair(ps, s, q, pp, r1pl, defer_dr):
            hdat = []
            for h in range(2):
                j = q * 4 + pp * 2 + h
                r0 = s + j * 128
                js = slice(j * 128, (j + 1) * 128)      # chunk-local
                rs = slice(r0, r0 + 128)                # shard-absolute
                hdat.append(rs)
                if defer_dr:
                    # early chunks: defer DoubleRow MMs until r23 lands
                    mm_lin(ps, h, r0, 0, True)
                    mm_lin(ps, h, r0, 1, False)
                    mm_bias(ps, h)
                    mm_r1(ps, h, r1pl, js, 0, False)
                    mm_r1(ps, h, r1pl, js, 1, False)
                else:
                    # steady state: interleave the two DoubleRow MMs
                    # (213ns LDWEIGHTS) under the N=512 linear MMs
                    mm_lin(ps, h, r0, 0, True)
                    mm_dr(ps, h, rs, 0, False)
                    mm_lin(ps, h, r0, 1, False)
                    mm_dr(ps, h, rs, 1, False)
                    mm_bias(ps, h)
                    mm_r1(ps, h, r1pl, js, 0, False)
                    mm_r1(ps, h, r1pl, js, 1, True)
            if defer_dr:
                for h in range(2):
                    mm_dr(ps, h, hdat[h], 0, False)
                    mm_dr(ps, h, hdat[h], 1, True)

        def epilogue(oq, sl, ps):
            nc.scalar.activation(
                out=oq[:, sl, :], in_=ps[:, :, 256:512], func=_ACT)
            nc.vector.tensor_tensor(
                out=oq[:, sl, :], in0=oq[:, sl, :], in1=ps[:, :, 0:256],
                op=mybir.AluOpType.add)

        for ci, (s, n) in enumerate(CHUNKS):
            cs = slice(s, s + n)
            # r1 = relu(tc + 1): one fused DVE op per chunk (fp16 4x mode)
            r1pl = planes_p.tile([128, 2, n], DT, tag=f"r1_{n}",
                                 name=f"r1_{ci}")
            nc.vector.tensor_scalar(
                out=r1pl[:], in0=xt[:, :, cs],
                scalar1=1.0, scalar2=0.0,
                op0=mybir.AluOpType.add, op1=mybir.AluOpType.max)
            last_chunk = ci == len(CHUNKS) - 1
            if n == 256:
                # tiny head chunk: one PSUM pair, pair-sized store
                ps = psM.tile([128, 2, 512], F32)
                do_pair(ps, s, 0, 0, r1pl, defer_dr=True)
                oq = outp.tile([128, 2, OUT_F], DT, tag="oq2")
                epilogue(oq, slice(0, 2), ps)
                nc.scalar.dma_start(
                    out=out_sh[s:s + 256, :].rearrange("(g p) n -> p g n", g=2),
                    in_=oq[:])
                continue
            n_quads = n // 512
            for q in range(n_quads):
                # four 128-row tiles -> one SBUF out tile; PSUM pairs
                oq = outp.tile([128, 4, OUT_F], DT, tag="oq4")
                q0 = s + q * 512
                last_quad = last_chunk and q == n_quads - 1
                for pp in range(2):
                    ps = psM.tile([128, 2, 512], F32)
                    do_pair(ps, s, q, pp, r1pl, defer_dr=False)
                    sl = slice(pp * 2, pp * 2 + 2)
                    if last_quad and pp == 1:
                        # final pair: single-tile epilogues + stores so the
                        # post-matmul tail is as short as possible
                        for h in range(2):
                            shl = slice(pp * 2 + h, pp * 2 + h + 1)
                            nc.scalar.activation(
                                out=oq[:, shl, :], in_=ps[:, h:h + 1, 256:512],
                                func=_ACT)
                            nc.vector.tensor_tensor(
                                out=oq[:, shl, :], in0=oq[:, shl, :],
                                in1=ps[:, h:h + 1, 0:256],
                                op=mybir.AluOpType.add)
                            r0t = q0 + pp * 256 + h * 128
                            nc.scalar.dma_start(
                                out=out_sh[r0t:r0t + 128, :],
                                in_=oq[:, pp * 2 + h, :])
                        continue
                    epilogue(oq, sl, ps)
                    if last_quad:
                        # split the final store per pair to shorten the tail
                        nc.scalar.dma_start(
                            out=out_sh[q0 + pp * 256:q0 + pp * 256 + 256, :]
                                .rearrange("(g p) n -> p g n", g=2),
                            in_=oq[:, sl, :])
                if not last_quad:
                    nc.scalar.dma_start(
                        out=out_sh[q0:q0 + 512, :].rearrange("(g p) n -> p g n", g=4),
                        in_=oq[:])
    nc.compile()
    return nc


_CACHE = {}


def make_in_maps(inputs):
    tc, w_pack, wr8, bias_row, C_host = _host_prep(**inputs)
    _CACHE["C_host"] = C_host
    maps = []
    for c in range(N_CORES):
        sh = tc[c * N_SHARD:(c + 1) * N_SHARD]          # [4096, 256]
        tct = np.ascontiguousarray(sh.T).reshape(2, 128, N_SHARD)
        t32 = tct.astype(np.float32)
        r23 = np.stack([np.maximum(t32, 0.0),           # r2 = relu(t-2)
                        np.maximum(t32 - 1.0, 0.0)])    # r3 = relu(t-3)
        maps.append({
            "tc_sh": tct, "r23_sh": r23.astype(NP_F8),
            "w_pack": w_pack, "wr8": np.ascontiguousarray(
                wr8.transpose(0, 2, 1, 3)),             # [m,128,b,256]
            "bias_row": bias_row,
        })
    return maps


def kernel(**inputs):
    if "nc" not in _CACHE:
        _CACHE["nc"] = _build_bass()
    nc = _CACHE["nc"]
    in_maps = make_in_maps(inputs)
    res = run_bass_kernel_spmd(nc, in_maps, list(range(N_CORES)))
    out = np.concatenate([res.results[c]["out_sh"] for c in range(N_CORES)], axis=0)
    out = out.astype(np.float32) + _CACHE["C_host"][None, :]
    return out.reshape(B, H, W, OUT_F)
